# revision 37
# baseline (speedup 1.0000x reference)
"""Trainium2 Bass kernel for nn_CViTFlow (cross-attention ViT flow block).

Math (per the module):
  two token streams x1,x2 [B,T,256] viewed as [B,256,48,48] images.
  6 branches (q1,k1,v1,q2,k2,v2): depthwise3x3 -> BN(eval) -> 1x1 conv -> Linear.
  o1 = softmax(-(q1 k2^T / 16)) v2 + q1 ;  o2 = softmax(-(q2 k1^T / 16)) v1 + q2
  both reshaped [B,H,T,DH] -> [B,T,256] with a plain (head-major) reshape.

Kernel strategy:
  * Host folds BN + 1x1conv + Linear into one 256x256 matrix W and bias c per
    branch, then folds the depthwise 3x3 into 9 "tap" matrices
    Wtap[o,c] = W[o,c]*dw[c,di,dj], so a whole branch becomes 9 shifted
    matmuls accumulated in PSUM (all compute lands on the TensorEngine).
  * Host pre-transposes/pads images to channel-major [256, 50, 50] (zero pad)
    so tap shifts are plain strided access patterns.
  * 8 cores = (map m) x (batch b) x (head-quad g): each core computes one
    output map's 4 heads for one batch element. No collectives.
  * On device: branch matmuls produce qT/kT/vT [128=4*32, T]; v is
    PE-transposed to [t,d] tiles augmented with a ones column (so the AV
    matmul also produces the softmax denominator); scores are computed
    t-chunk-wise with 4 row-tiled (K=32) matmuls sharing the PE array;
    exp on ScalarE; AV accumulates over t in PSUM; finalize divides by the
    denominator (reciprocal + PE broadcast) and adds the q residual.
"""

import numpy as np

B = 2
T = 2304
DIM = 256
HEADS = 8
DH = 32
HW = 48
EPS = 1e-5
P = 128
N_CORES = 8

# t-tiles for the branch phase: row-aligned in the 48x48 image (10/8 rows)
T_TILES = [(0, 480, 0, 10), (480, 480, 10, 10), (960, 480, 20, 10),
           (1440, 480, 30, 10), (1920, 384, 40, 8)]
# l-tiles for the attention phase (256 wide: scores PSUM [128,1024] = 2 banks,
# double-buffered, leaving 4 banks for the col-packed AV accumulators)
NL = 256
N_LT = T // NL  # 9
N_TCH = T // P  # 18 t-chunks of 128 for scores/AV

_PROGRAM = None  # cached Bass program
_last_in_maps = None  # stashed per-core input maps (for external profiling runs)


def _build_program(debug=False):
    """Build the SPMD Bass/Tile program (identical for all 8 cores)."""
    from contextlib import ExitStack

    import concourse.bacc as bacc
    import concourse.mybir as mybir
    import concourse.tile as tile
    from concourse.masks import make_identity
    from concourse.tile_rust import add_dep_helper

    f32 = mybir.dt.float32
    bf16 = mybir.dt.bfloat16
    AF = mybir.ActivationFunctionType
    OP = mybir.AluOpType

    # Bacc (not raw Bass): its compile() runs move_matmul_waits_to_ldweights +
    # generate_event_semaphores, without which walrus rejects multi-wait matmuls
    nc = bacc.Bacc(None, target_bir_lowering=False, debug=False)

    # DRAM I/O (per core).  pad_a feeds the q branch, pad_b feeds k and v.
    # Matmul operands are bf16 (fp32 matmuls cost 2 PE passes via LOW_HIGH
    # mode); accumulation stays fp32 in PSUM.
    pad_a = nc.declare_dram_parameter("pad_a", [2, P, 2500], bf16, isOutput=False)
    pad_b = nc.declare_dram_parameter("pad_b", [2, P, 2500], bf16, isOutput=False)
    wq = nc.declare_dram_parameter("wq", [2, P, 9 * P], bf16, isOutput=False)
    wk = nc.declare_dram_parameter("wk", [2, P, 9 * P], bf16, isOutput=False)
    wv = nc.declare_dram_parameter("wv", [2, P, 9 * P], bf16, isOutput=False)
    bias_d = nc.declare_dram_parameter("bias", [3, P, 1], f32, isOutput=False)
    out_d = nc.declare_dram_parameter("out", [P, T], f32, isOutput=True)

    with tile.TileContext(nc) as tc, ExitStack() as ctx:
        const = ctx.enter_context(tc.tile_pool(name="const", bufs=1))
        sb = ctx.enter_context(tc.tile_pool(name="sb", bufs=1))
        fin = ctx.enter_context(tc.tile_pool(name="fin", bufs=2))

        identity = const.tile([P, P], bf16)
        make_identity(nc, identity)
        ones32 = const.tile([1, 32], f32)
        nc.vector.memset(ones32[:], 1.0)

        # ---- input DMAs (k/v weights + image B first: they gate phase A) ----
        wk_sb = sb.tile([P, 2 * 9 * P], bf16, tag="wk")
        pb_sb = sb.tile([P, 2 * 2500], bf16, tag="pb")
        wv_sb = sb.tile([P, 2 * 9 * P], bf16, tag="wv")
        wq_sb = sb.tile([P, 2 * 9 * P], bf16, tag="wq")
        pa_sb = sb.tile([P, 2 * 2500], bf16, tag="pa")
        bias_sb = sb.tile([P, 3], f32, tag="bias")
        for kc in range(2):
            nc.sync.dma_start(wk_sb[:, kc * 1152:(kc + 1) * 1152], wk[kc])
            nc.sync.dma_start(pb_sb[:, kc * 2500:(kc + 1) * 2500], pad_b[kc])
        for kc in range(2):
            nc.sync.dma_start(wv_sb[:, kc * 1152:(kc + 1) * 1152], wv[kc])
        for kc in range(2):
            nc.sync.dma_start(wq_sb[:, kc * 1152:(kc + 1) * 1152], wq[kc])
            nc.sync.dma_start(pa_sb[:, kc * 2500:(kc + 1) * 2500], pad_a[kc])
        for r in range(3):
            nc.sync.dma_start(bias_sb[:, r:r + 1], bias_d[r])

        qT = sb.tile([P, T], bf16, tag="qT")
        qTf = sb.tile([P, T], f32, tag="qTf")   # fp32 copy for the residual
        kT = sb.tile([P, T], bf16, tag="kT")
        vT = sb.tile([P, T], bf16, tag="vT")
        # base-partition-0 copies (head h at cols h*T): score matmuls then all
        # use row strip 0 -> strict-FIFO on PE, so no two concurrent matmul
        # drains ever target the same PSUM bank (same-partition same-bank
        # concurrent drains crash the device)
        qT2 = sb.tile([32, 4 * T], bf16, tag="qT2")
        kT2 = sb.tile([32, 4 * T], bf16, tag="kT2")
        # per t-chunk, per head: 64 cols = [v(32) | ones(1) | zeros(31)] so AV
        # pairs col-tile at (0,0)/(0,64) and the ones column carries the
        # softmax denominator through the same matmul
        vaug = sb.tile([P, N_TCH * 256], bf16, tag="vaug")
        outbuf = sb.tile([P, T], f32, tag="outbuf")

        # ================= Phase A: branch matmuls =================
        def branch(psumA, w_sb, img_sb, dest, role, dest2=None, rebase=None):
            for (t0, nt, r0, nr) in T_TILES:
                ps = psumA.tile([P, nt], f32, tag="sc", bufs=2,
                                name=f"br_{role}_{t0}")
                mm = 0
                for kc in range(2):
                    pv = img_sb[:, kc * 2500:(kc + 1) * 2500].rearrange(
                        "p (r c) -> p r c", c=50)
                    wv_ = w_sb[:, kc * 1152:(kc + 1) * 1152]
                    for di in range(3):
                        for dj in range(3):
                            tap = di * 3 + dj
                            rhs = pv[:, r0 + di:r0 + di + nr, dj:dj + 48]
                            nc.tensor.matmul(
                                ps[:], wv_[:, tap * P:(tap + 1) * P], rhs,
                                start=(mm == 0), stop=(mm == 17))
                            mm += 1
                # bias add, PSUM -> SBUF (bf16 for matmul operands)
                nc.vector.tensor_scalar_add(dest[:, t0:t0 + nt], ps[:],
                                            bias_sb[:, role:role + 1])
                if dest2 is not None:
                    nc.vector.tensor_scalar_add(dest2[:, t0:t0 + nt], ps[:],
                                                bias_sb[:, role:role + 1])
                if rebase is not None:
                    # rebase this slice to partition 0 (head h at cols h*T)
                    # via SBUF->SBUF DMA, spread through the phase so the PE
                    # never has a long DMA-wait gap at the phase transition
                    for h in range(4):
                        nc.sync.dma_start(
                            rebase[0:32, h * T + t0:h * T + t0 + nt],
                            dest[h * 32:(h + 1) * 32, t0:t0 + nt])

        # One PSUM pool for the whole kernel (a pool transition emits barrier
        # boundaries -> a PE idle gap that drops the HAM clock to 1.2 GHz for
        # the rest of the kernel). Branch tiles borrow the "sc" slots, the
        # v-transposes borrow the "avout" slots.
        psumB = ctx.enter_context(tc.tile_pool(name="psum", bufs=2, space="PSUM"))
        ep = ctx.enter_context(tc.tile_pool(name="ep", bufs=2))

        branch(psumB, wk_sb, pb_sb, kT, 1, rebase=kT2)
        branch(psumB, wv_sb, pb_sb, vT, 2)
        branch(psumB, wq_sb, pa_sb, qT, 0, dest2=qTf, rebase=qT2)

        # v: transpose to [t, d] tiles, 64 cols per head
        nc.vector.memset(vaug[:], 0.0)
        for j in range(N_TCH):
            tp = psumB.tile([P, 512], bf16, tag="avout", bufs=2, name=f"tp_{j}")
            nc.tensor.transpose(tp[:, 0:P], vT[:, j * P:(j + 1) * P], identity[:])
            dst = vaug[:, j * 256:(j + 1) * 256].rearrange(
                "p (h c) -> p h c", c=64)[:, :, 0:32]
            src = tp[:, 0:P].rearrange("p (h c) -> p h c", c=32)
            nc.vector.tensor_copy(dst, src)
        ones_cols = vaug.rearrange("p (j h c) -> p j h c", h=4, c=64)[:, :, :, 32:33]
        nc.vector.memset(ones_cols, 1.0)

        # ================= Phase B: attention =================
        # Software-pipelined so the in-order PE stream never sits behind an
        # exp wait: scores(t+1) issue before AV(t); finalize(l-1) is injected
        # mid-way through l's t-loop (its bc matmuls wait on DVE reciprocals).

        sc_tiles = {}

        def scores(li, j):
            l0 = li * NL
            sc = psumB.tile([P, 4 * NL], f32, tag="sc", bufs=2,
                            name=f"sc_{li}_{j}")
            last = None
            for h in range(4):
                last = nc.tensor.matmul(
                    sc[:, h * NL:(h + 1) * NL],
                    kT2[0:32, h * T + j * P:h * T + (j + 1) * P],
                    qT2[0:32, h * T + l0:h * T + l0 + NL],
                    start=True, stop=True)
            sc_tiles[(li, j)] = (sc, last)

        def finalize(li, outp):
            l0 = li * NL
            bc_ps = psumB.tile([32, 4 * NL], f32, tag="sc", bufs=2,
                               name=f"bc_{li}")
            for h in range(4):
                cp, pb_ = 256 * (h // 2), 64 * (h % 2)
                recip = fin.tile([1, NL], f32, tag="recip")
                nc.vector.reciprocal(recip[:],
                                     outp[pb_ + 32:pb_ + 33, cp:cp + NL])
                nc.tensor.matmul(bc_ps[:, h * NL:(h + 1) * NL],
                                 ones32[:], recip[:], start=True, stop=True)
                bc_sb = fin.tile([32, NL], f32, tag="bcsb")
                nc.vector.tensor_copy(bc_sb[:], bc_ps[:, h * NL:(h + 1) * NL])
                # av * (1/denom): PSUM+SBUF inputs may differ in base
                # partition (only SB+SB pairs must match), out lands at 32h
                nc.vector.tensor_tensor(outbuf[h * 32:(h + 1) * 32, l0:l0 + NL],
                                        outp[pb_:pb_ + 32, cp:cp + NL],
                                        bc_sb[:], op=OP.mult)
                # in-place residual: both SB inputs at base partition 32h
                nc.vector.tensor_tensor(outbuf[h * 32:(h + 1) * 32, l0:l0 + NL],
                                        outbuf[h * 32:(h + 1) * 32, l0:l0 + NL],
                                        qTf[h * 32:(h + 1) * 32, l0:l0 + NL],
                                        op=OP.add)

        prev = None  # (li, outp) awaiting finalize
        for li in range(N_LT):
            # one bank: pair p at cols 256p, sub s at partitions 64s
            outp = psumB.tile([P, 2 * NL], f32, tag="avout", bufs=2,
                              name=f"avout_{li}")
            if li == 0:
                scores(0, 0)
            for j in range(N_TCH):
                if j + 1 < N_TCH:
                    scores(li, j + 1)
                elif li + 1 < N_LT:
                    scores(li + 1, 0)
                next_last = (sc_tiles[(li, j + 1)][1] if (li, j + 1) in sc_tiles
                             else sc_tiles.get((li + 1, 0), (None, None))[1])
                sc, _ = sc_tiles.pop((li, j))
                et = ep.tile([P, 4 * NL], bf16, tag="e")
                nc.scalar.activation(et[:], sc[:], AF.Exp, scale=-0.0625)
                for h in range(4):
                    cp, sub = 256 * (h // 2), h % 2
                    av = nc.tensor.matmul(
                        outp[64 * sub:64 * sub + 64, cp:cp + NL],
                        vaug[:, j * 256 + 64 * h:j * 256 + 64 * h + 64],
                        et[:, h * NL:(h + 1) * NL],
                        start=(j == 0), stop=(j == N_TCH - 1),
                        tile_position=(0, 64 * sub),
                        skip_group_check=True)
                    if h == 0 and next_last is not None:
                        # keep next scores AHEAD of this exp-gated AV in the
                        # in-order PE stream (scheduling-only ordering edge)
                        add_dep_helper(av.ins, next_last.ins,
                                       reason="scores(t+1) before AV(t)")
                if j == 8 and prev is not None:
                    finalize(*prev)
                    prev = None
            prev = (li, outp)
        finalize(*prev)

        nc.sync.dma_start(out_d[:], outbuf[:])

        if debug:
            for nm, t in [("dbg_qT", qT), ("dbg_kT", kT), ("dbg_vT", vT),
                          ("dbg_vaug", vaug), ("dbg_qTf", qTf)]:
                dd = nc.declare_dram_parameter(nm, list(t.shape), t.dtype,
                                               isOutput=True)
                nc.sync.dma_start(dd[:], t[:])

    nc.compile()
    return nc


def _fold_weights(dw_w, bn_gamma, bn_beta, bn_mean, bn_var, pw_w, pw_b, lin_w):
    """Fold BN + pointwise conv + linear (+ depthwise taps) per branch.

    Returns Wtap [6, 9, 256, 256] (float32) and bias c [6, 256]."""
    dw = dw_w.astype(np.float64)
    g = bn_gamma.astype(np.float64)
    b = bn_beta.astype(np.float64)
    m = bn_mean.astype(np.float64)
    v = bn_var.astype(np.float64)
    pw = pw_w.astype(np.float64)
    pb = pw_b.astype(np.float64)
    lw = lin_w.astype(np.float64)

    scale = g / np.sqrt(v + EPS)                      # [6, 256]
    shift = b - m * scale                             # [6, 256]
    M = np.einsum("noc,ncd->nod", lw, pw)             # lin @ pw  [6, 256, 256]
    W = M * scale[:, None, :]                         # [6, 256(o), 256(c)]
    c = np.einsum("noc,nc->no", M, shift) + np.einsum("noc,nc->no", lw, pb)
    # taps: Wtap[n, di*3+dj, o, c] = W[n, o, c] * dw[n, c, di, dj]
    Wtap = W[:, None, :, :] * dw.transpose(0, 2, 3, 1).reshape(6, 9, 1, 256)
    return Wtap.astype(np.float32), c.astype(np.float32)


def _bf16(a):
    import ml_dtypes
    return a.astype(ml_dtypes.bfloat16)


def _pad_images(x):
    """x [B, T, 256] -> per batch channel-major zero-padded bf16 [2,128,2500]."""
    out = np.zeros((B, 2, P, 50, 50), dtype=np.float32)
    img = np.ascontiguousarray(x.transpose(0, 2, 1)).reshape(B, DIM, HW, HW)
    out[:, :, :, 1:49, 1:49] = img.reshape(B, 2, P, HW, HW)
    return _bf16(out.reshape(B, 2, P, 2500))


def _wtap_lhsT(Wtap, branch, g):
    """Pack lhsT layout [2, 128, 9*128] for a branch restricted to quad g."""
    rows = slice(g * P, (g + 1) * P)
    out = np.empty((2, P, 9 * P), dtype=np.float32)
    for kc in range(2):
        for tap in range(9):
            blk = Wtap[branch, tap][rows, kc * P:(kc + 1) * P]  # [128 o, 128 c]
            out[kc, :, tap * P:(tap + 1) * P] = blk.T
    return _bf16(out)


def kernel(x1, x2, dw_w, bn_gamma, bn_beta, bn_mean, bn_var, pw_w, pw_b, lin_w,
           h1=HW, w1=HW, h2=HW, w2=HW):
    global _PROGRAM
    from concourse.bass_utils import run_bass_kernel_spmd

    x1 = np.asarray(x1, dtype=np.float32)
    x2 = np.asarray(x2, dtype=np.float32)

    Wtap, c = _fold_weights(np.asarray(dw_w), np.asarray(bn_gamma),
                            np.asarray(bn_beta), np.asarray(bn_mean),
                            np.asarray(bn_var), np.asarray(pw_w),
                            np.asarray(pw_b), np.asarray(lin_w))
    pad1 = _pad_images(x1)   # [B, 2, 128, 2500]
    pad2 = _pad_images(x2)

    if _PROGRAM is None:
        _PROGRAM = _build_program()
    nc = _PROGRAM

    # core layout: core = m*4 + b*2 + g
    # map m=0: o1 = att(q=br0(x1), k=br4(x2), v=br5(x2)) + q1
    # map m=1: o2 = att(q=br3(x2), k=br1(x1), v=br2(x1)) + q2
    in_maps = []
    for m in range(2):
        qbr, kbr, vbr = (0, 4, 5) if m == 0 else (3, 1, 2)
        pa, pb_ = (pad1, pad2) if m == 0 else (pad2, pad1)
        for b in range(2):
            for g in range(2):
                bias = np.stack([c[qbr, g * P:(g + 1) * P],
                                 c[kbr, g * P:(g + 1) * P],
                                 c[vbr, g * P:(g + 1) * P]])[:, :, None]
                in_maps.append({
                    "pad_a": np.ascontiguousarray(pa[b]),
                    "pad_b": np.ascontiguousarray(pb_[b]),
                    "wq": _wtap_lhsT(Wtap, qbr, g),
                    "wk": _wtap_lhsT(Wtap, kbr, g),
                    "wv": _wtap_lhsT(Wtap, vbr, g),
                    "bias": np.ascontiguousarray(bias),
                })

    global _last_in_maps
    _last_in_maps = in_maps
    res = run_bass_kernel_spmd(nc, in_maps, list(range(N_CORES)))

    o = np.empty((2, 2, HEADS, T, DH), dtype=np.float32)
    for m in range(2):
        for b in range(2):
            for g in range(2):
                core = m * 4 + b * 2 + g
                blk = res.results[core]["out"].reshape(4, DH, T)
                o[m, b, 4 * g:4 * g + 4] = blk.transpose(0, 2, 1)
    o1 = o[0].reshape(B, T, HEADS * DH)
    o2 = o[1].reshape(B, T, HEADS * DH)
    return o1, o2


# revision 39
# speedup vs baseline: 1.2971x; 1.2971x over previous
"""Trainium2 Bass kernel for nn_CViTFlow (cross-attention ViT flow block).

Math (per the module):
  two token streams x1,x2 [B,T,256] viewed as [B,256,48,48] images.
  6 branches (q1,k1,v1,q2,k2,v2): depthwise3x3 -> BN(eval) -> 1x1 conv -> Linear.
  o1 = softmax(-(q1 k2^T / 16)) v2 + q1 ;  o2 = softmax(-(q2 k1^T / 16)) v1 + q2
  both reshaped [B,H,T,DH] -> [B,T,256] with a plain (head-major) reshape.

Kernel strategy:
  * Host folds BN + 1x1conv + Linear into one 256x256 matrix W and bias c per
    branch, then folds the depthwise 3x3 into 9 "tap" matrices
    Wtap[o,c] = W[o,c]*dw[c,di,dj], so a whole branch becomes 9 shifted
    matmuls accumulated in PSUM (all compute lands on the TensorEngine).
  * Host pre-transposes/pads images to channel-major [256, 50, 50] (zero pad)
    so tap shifts are plain strided access patterns.
  * 8 cores = (map m) x (batch b) x (head-quad g): each core computes one
    output map's 4 heads for one batch element. No collectives.
  * On device: branch matmuls produce qT/kT/vT [128=4*32, T]; v is
    PE-transposed to [t,d] tiles augmented with a ones column (so the AV
    matmul also produces the softmax denominator); scores are computed
    t-chunk-wise with 4 row-tiled (K=32) matmuls sharing the PE array;
    exp on ScalarE; AV accumulates over t in PSUM; finalize divides by the
    denominator (reciprocal + PE broadcast) and adds the q residual.
"""

import numpy as np

B = 2
T = 2304
DIM = 256
HEADS = 8
DH = 32
HW = 48
EPS = 1e-5
P = 128
N_CORES = 8

# t-tiles for the branch phase: row-aligned in the 48x48 image (10/8 rows)
T_TILES = [(0, 480, 0, 10), (480, 480, 10, 10), (960, 480, 20, 10),
           (1440, 480, 30, 10), (1920, 384, 40, 8)]
# l-tiles for the attention phase (256 wide: scores PSUM [128,1024] = 2 banks,
# double-buffered, leaving 4 banks for the col-packed AV accumulators)
NL = 256
N_LT = T // NL  # 9
N_TCH = T // P  # 18 t-chunks of 128 for scores/AV

_PROGRAM = None  # cached Bass program
_last_in_maps = None  # stashed per-core input maps (for external profiling runs)


def _build_program(debug=False):
    """Build the SPMD Bass/Tile program (identical for all 8 cores)."""
    from contextlib import ExitStack

    import concourse.bacc as bacc
    import concourse.mybir as mybir
    import concourse.tile as tile
    from concourse.masks import make_identity
    from concourse.tile_rust import add_dep_helper

    f32 = mybir.dt.float32
    bf16 = mybir.dt.bfloat16
    AF = mybir.ActivationFunctionType
    OP = mybir.AluOpType

    # Bacc (not raw Bass): its compile() runs move_matmul_waits_to_ldweights +
    # generate_event_semaphores, without which walrus rejects multi-wait matmuls
    nc = bacc.Bacc(None, target_bir_lowering=False, debug=False)

    # DRAM I/O (per core).  pad_a feeds the q branch, pad_b feeds k and v.
    # Matmul operands are bf16 (fp32 matmuls cost 2 PE passes via LOW_HIGH
    # mode); accumulation stays fp32 in PSUM.
    pad_a = nc.declare_dram_parameter("pad_a", [2, P, 2500], bf16, isOutput=False)
    pad_b = nc.declare_dram_parameter("pad_b", [2, P, 2500], bf16, isOutput=False)
    wq = nc.declare_dram_parameter("wq", [2, P, 9 * P], bf16, isOutput=False)
    wk = nc.declare_dram_parameter("wk", [2, P, 9 * P], bf16, isOutput=False)
    wv = nc.declare_dram_parameter("wv", [2, P, 9 * P], bf16, isOutput=False)
    bias_d = nc.declare_dram_parameter("bias", [3, P, 1], f32, isOutput=False)
    out_d = nc.declare_dram_parameter("out", [P, T], f32, isOutput=True)

    with tile.TileContext(nc) as tc, ExitStack() as ctx:
        const = ctx.enter_context(tc.tile_pool(name="const", bufs=1))
        sb = ctx.enter_context(tc.tile_pool(name="sb", bufs=1))
        fin = ctx.enter_context(tc.tile_pool(name="fin", bufs=2))

        identity = const.tile([P, P], bf16)
        make_identity(nc, identity)
        ones32 = const.tile([1, 32], f32)
        nc.vector.memset(ones32[:], 1.0)

        # ---- input DMAs (k/v weights + image B first: they gate phase A) ----
        wk_sb = sb.tile([P, 2 * 9 * P], bf16, tag="wk")
        pb_sb = sb.tile([P, 2 * 2500], bf16, tag="pb")
        wv_sb = sb.tile([P, 2 * 9 * P], bf16, tag="wv")
        wq_sb = sb.tile([P, 2 * 9 * P], bf16, tag="wq")
        pa_sb = sb.tile([P, 2 * 2500], bf16, tag="pa")
        bias_sb = sb.tile([P, 3], f32, tag="bias")
        for kc in range(2):
            nc.sync.dma_start(wk_sb[:, kc * 1152:(kc + 1) * 1152], wk[kc])
            nc.sync.dma_start(pb_sb[:, kc * 2500:(kc + 1) * 2500], pad_b[kc])
        for kc in range(2):
            nc.sync.dma_start(wv_sb[:, kc * 1152:(kc + 1) * 1152], wv[kc])
        for kc in range(2):
            nc.sync.dma_start(wq_sb[:, kc * 1152:(kc + 1) * 1152], wq[kc])
            nc.sync.dma_start(pa_sb[:, kc * 2500:(kc + 1) * 2500], pad_a[kc])
        for r in range(3):
            nc.sync.dma_start(bias_sb[:, r:r + 1], bias_d[r])

        qT = sb.tile([P, T], bf16, tag="qT")
        qTf = sb.tile([P, T], f32, tag="qTf")   # fp32 copy for the residual
        kT = sb.tile([P, T], bf16, tag="kT")
        vT = sb.tile([P, T], bf16, tag="vT")
        # base-partition-0 copies (head h at cols h*T): score matmuls then all
        # use row strip 0 -> strict-FIFO on PE, so no two concurrent matmul
        # drains ever target the same PSUM bank (same-partition same-bank
        # concurrent drains crash the device)
        qT2 = sb.tile([32, 4 * T], bf16, tag="qT2")
        kT2 = sb.tile([32, 4 * T], bf16, tag="kT2")
        # per t-chunk, per head: 64 cols = [v(32) | ones(1) | zeros(31)] so AV
        # pairs col-tile at (0,0)/(0,64) and the ones column carries the
        # softmax denominator through the same matmul
        vaug = sb.tile([P, N_TCH * 256], bf16, tag="vaug")
        outbuf = sb.tile([P, T], f32, tag="outbuf")

        # ================= Phase A: branch matmuls =================
        def branch(psumA, w_sb, img_sb, dest, role, dest2=None, rebase=None):
            for (t0, nt, r0, nr) in T_TILES:
                ps = psumA.tile([P, nt], f32, tag="sc", bufs=2,
                                name=f"br_{role}_{t0}")
                mm = 0
                for kc in range(2):
                    pv = img_sb[:, kc * 2500:(kc + 1) * 2500].rearrange(
                        "p (r c) -> p r c", c=50)
                    wv_ = w_sb[:, kc * 1152:(kc + 1) * 1152]
                    for di in range(3):
                        for dj in range(3):
                            tap = di * 3 + dj
                            rhs = pv[:, r0 + di:r0 + di + nr, dj:dj + 48]
                            nc.tensor.matmul(
                                ps[:], wv_[:, tap * P:(tap + 1) * P], rhs,
                                start=(mm == 0), stop=(mm == 17))
                            mm += 1
                # bias add, PSUM -> SBUF (bf16 for matmul operands)
                nc.vector.tensor_scalar_add(dest[:, t0:t0 + nt], ps[:],
                                            bias_sb[:, role:role + 1])
                if dest2 is not None:
                    nc.vector.tensor_scalar_add(dest2[:, t0:t0 + nt], ps[:],
                                                bias_sb[:, role:role + 1])
                if rebase is not None:
                    # rebase this slice to partition 0 (head h at cols h*T)
                    # via SBUF->SBUF DMA, spread through the phase so the PE
                    # never has a long DMA-wait gap at the phase transition
                    for h in range(4):
                        nc.sync.dma_start(
                            rebase[0:32, h * T + t0:h * T + t0 + nt],
                            dest[h * 32:(h + 1) * 32, t0:t0 + nt])

        # One PSUM pool for the whole kernel (a pool transition emits barrier
        # boundaries -> a PE idle gap that drops the HAM clock to 1.2 GHz for
        # the rest of the kernel). Branch tiles borrow the "sc" slots, the
        # v-transposes borrow the "avout" slots.
        psumB = ctx.enter_context(tc.tile_pool(name="psum", bufs=2, space="PSUM"))
        ep = ctx.enter_context(tc.tile_pool(name="ep", bufs=2))

        branch(psumB, wk_sb, pb_sb, kT, 1, rebase=kT2)
        branch(psumB, wv_sb, pb_sb, vT, 2)
        branch(psumB, wq_sb, pa_sb, qT, 0, dest2=qTf, rebase=qT2)

        # v: transpose to [t, d] tiles, 64 cols per head
        nc.vector.memset(vaug[:], 0.0)
        for j in range(N_TCH):
            tp = psumB.tile([P, 512], bf16, tag="avout", bufs=2, name=f"tp_{j}")
            nc.tensor.transpose(tp[:, 0:P], vT[:, j * P:(j + 1) * P], identity[:])
            dst = vaug[:, j * 256:(j + 1) * 256].rearrange(
                "p (h c) -> p h c", c=64)[:, :, 0:32]
            src = tp[:, 0:P].rearrange("p (h c) -> p h c", c=32)
            nc.vector.tensor_copy(dst, src)
        ones_cols = vaug.rearrange("p (j h c) -> p j h c", h=4, c=64)[:, :, :, 32:33]
        nc.vector.memset(ones_cols, 1.0)

        # ================= Phase B: attention =================
        # Software-pipelined so the in-order PE stream never sits behind an
        # exp wait: scores(t+1) issue before AV(t); finalize(l-1) is injected
        # mid-way through l's t-loop (its bc matmuls wait on DVE reciprocals).

        sc_tiles = {}

        def scores(li, j):
            l0 = li * NL
            sc = psumB.tile([P, 4 * NL], f32, tag="sc", bufs=2,
                            name=f"sc_{li}_{j}")
            last = None
            for h in range(4):
                last = nc.tensor.matmul(
                    sc[:, h * NL:(h + 1) * NL],
                    kT2[0:32, h * T + j * P:h * T + (j + 1) * P],
                    qT2[0:32, h * T + l0:h * T + l0 + NL],
                    start=True, stop=True)
            sc_tiles[(li, j)] = (sc, last)

        def finalize_recips(li, outp):
            # issue right after the l-tile's AV accumulation completes: the
            # reciprocals (8 cyc/elem on DVE) then overlap the next l-tile's
            # attention instead of head-of-line-blocking the PE at the bc MMs
            recips = []
            for h in range(4):
                cp, pb_ = 256 * (h // 2), 64 * (h % 2)
                recip = fin.tile([1, NL], f32, tag="recip", bufs=8,
                                 name=f"recip_{li}_{h}")
                nc.vector.reciprocal(recip[:],
                                     outp[pb_ + 32:pb_ + 33, cp:cp + NL])
                recips.append(recip)
            return recips

        def finalize(li, outp, recips):
            l0 = li * NL
            bc_ps = psumB.tile([32, 4 * NL], f32, tag="sc", bufs=2,
                               name=f"bc_{li}")
            for h in range(4):
                cp, pb_ = 256 * (h // 2), 64 * (h % 2)
                nc.tensor.matmul(bc_ps[:, h * NL:(h + 1) * NL],
                                 ones32[:], recips[h][:], start=True, stop=True)
                bc_sb = fin.tile([32, NL], f32, tag="bcsb")
                nc.vector.tensor_copy(bc_sb[:], bc_ps[:, h * NL:(h + 1) * NL])
                # av * (1/denom): PSUM+SBUF inputs may differ in base
                # partition (only SB+SB pairs must match), out lands at 32h
                nc.vector.tensor_tensor(outbuf[h * 32:(h + 1) * 32, l0:l0 + NL],
                                        outp[pb_:pb_ + 32, cp:cp + NL],
                                        bc_sb[:], op=OP.mult)
                # in-place residual: both SB inputs at base partition 32h
                nc.vector.tensor_tensor(outbuf[h * 32:(h + 1) * 32, l0:l0 + NL],
                                        outbuf[h * 32:(h + 1) * 32, l0:l0 + NL],
                                        qTf[h * 32:(h + 1) * 32, l0:l0 + NL],
                                        op=OP.add)

        prev = None  # (li, outp) awaiting finalize
        for li in range(N_LT):
            # one bank: pair p at cols 256p, sub s at partitions 64s
            outp = psumB.tile([P, 2 * NL], f32, tag="avout", bufs=2,
                              name=f"avout_{li}")
            if li == 0:
                scores(0, 0)
            for j in range(N_TCH):
                if j + 1 < N_TCH:
                    scores(li, j + 1)
                elif li + 1 < N_LT:
                    scores(li + 1, 0)
                next_last = (sc_tiles[(li, j + 1)][1] if (li, j + 1) in sc_tiles
                             else sc_tiles.get((li + 1, 0), (None, None))[1])
                sc, _ = sc_tiles.pop((li, j))
                et = ep.tile([P, 4 * NL], bf16, tag="e")
                nc.scalar.activation(et[:], sc[:], AF.Exp, scale=-0.0625)
                for h in range(4):
                    cp, sub = 256 * (h // 2), h % 2
                    av = nc.tensor.matmul(
                        outp[64 * sub:64 * sub + 64, cp:cp + NL],
                        vaug[:, j * 256 + 64 * h:j * 256 + 64 * h + 64],
                        et[:, h * NL:(h + 1) * NL],
                        start=(j == 0), stop=(j == N_TCH - 1),
                        tile_position=(0, 64 * sub),
                        skip_group_check=True)
                    if h == 0 and next_last is not None:
                        # keep next scores AHEAD of this exp-gated AV in the
                        # in-order PE stream (scheduling-only ordering edge)
                        add_dep_helper(av.ins, next_last.ins,
                                       reason="scores(t+1) before AV(t)")
                if j == 8 and prev is not None:
                    finalize(*prev)
                    prev = None
            prev = (li, outp, finalize_recips(li, outp))
        finalize(*prev)

        nc.sync.dma_start(out_d[:], outbuf[:])

        if debug:
            for nm, t in [("dbg_qT", qT), ("dbg_kT", kT), ("dbg_vT", vT),
                          ("dbg_vaug", vaug), ("dbg_qTf", qTf)]:
                dd = nc.declare_dram_parameter(nm, list(t.shape), t.dtype,
                                               isOutput=True)
                nc.sync.dma_start(dd[:], t[:])

    nc.compile()
    return nc


def _fold_weights(dw_w, bn_gamma, bn_beta, bn_mean, bn_var, pw_w, pw_b, lin_w):
    """Fold BN + pointwise conv + linear (+ depthwise taps) per branch.

    Returns Wtap [6, 9, 256, 256] (float32) and bias c [6, 256]."""
    dw = dw_w.astype(np.float64)
    g = bn_gamma.astype(np.float64)
    b = bn_beta.astype(np.float64)
    m = bn_mean.astype(np.float64)
    v = bn_var.astype(np.float64)
    pw = pw_w.astype(np.float64)
    pb = pw_b.astype(np.float64)
    lw = lin_w.astype(np.float64)

    scale = g / np.sqrt(v + EPS)                      # [6, 256]
    shift = b - m * scale                             # [6, 256]
    M = np.einsum("noc,ncd->nod", lw, pw)             # lin @ pw  [6, 256, 256]
    W = M * scale[:, None, :]                         # [6, 256(o), 256(c)]
    c = np.einsum("noc,nc->no", M, shift) + np.einsum("noc,nc->no", lw, pb)
    # taps: Wtap[n, di*3+dj, o, c] = W[n, o, c] * dw[n, c, di, dj]
    Wtap = W[:, None, :, :] * dw.transpose(0, 2, 3, 1).reshape(6, 9, 1, 256)
    return Wtap.astype(np.float32), c.astype(np.float32)


def _bf16(a):
    import ml_dtypes
    return a.astype(ml_dtypes.bfloat16)


def _pad_images(x):
    """x [B, T, 256] -> per batch channel-major zero-padded bf16 [2,128,2500]."""
    out = np.zeros((B, 2, P, 50, 50), dtype=np.float32)
    img = np.ascontiguousarray(x.transpose(0, 2, 1)).reshape(B, DIM, HW, HW)
    out[:, :, :, 1:49, 1:49] = img.reshape(B, 2, P, HW, HW)
    return _bf16(out.reshape(B, 2, P, 2500))


def _wtap_lhsT(Wtap, branch, g):
    """Pack lhsT layout [2, 128, 9*128] for a branch restricted to quad g."""
    rows = slice(g * P, (g + 1) * P)
    out = np.empty((2, P, 9 * P), dtype=np.float32)
    for kc in range(2):
        for tap in range(9):
            blk = Wtap[branch, tap][rows, kc * P:(kc + 1) * P]  # [128 o, 128 c]
            out[kc, :, tap * P:(tap + 1) * P] = blk.T
    return _bf16(out)


def kernel(x1, x2, dw_w, bn_gamma, bn_beta, bn_mean, bn_var, pw_w, pw_b, lin_w,
           h1=HW, w1=HW, h2=HW, w2=HW):
    global _PROGRAM
    from concourse.bass_utils import run_bass_kernel_spmd

    x1 = np.asarray(x1, dtype=np.float32)
    x2 = np.asarray(x2, dtype=np.float32)

    Wtap, c = _fold_weights(np.asarray(dw_w), np.asarray(bn_gamma),
                            np.asarray(bn_beta), np.asarray(bn_mean),
                            np.asarray(bn_var), np.asarray(pw_w),
                            np.asarray(pw_b), np.asarray(lin_w))
    pad1 = _pad_images(x1)   # [B, 2, 128, 2500]
    pad2 = _pad_images(x2)

    if _PROGRAM is None:
        _PROGRAM = _build_program()
    nc = _PROGRAM

    # core layout: core = m*4 + b*2 + g
    # map m=0: o1 = att(q=br0(x1), k=br4(x2), v=br5(x2)) + q1
    # map m=1: o2 = att(q=br3(x2), k=br1(x1), v=br2(x1)) + q2
    in_maps = []
    for m in range(2):
        qbr, kbr, vbr = (0, 4, 5) if m == 0 else (3, 1, 2)
        pa, pb_ = (pad1, pad2) if m == 0 else (pad2, pad1)
        for b in range(2):
            for g in range(2):
                bias = np.stack([c[qbr, g * P:(g + 1) * P],
                                 c[kbr, g * P:(g + 1) * P],
                                 c[vbr, g * P:(g + 1) * P]])[:, :, None]
                in_maps.append({
                    "pad_a": np.ascontiguousarray(pa[b]),
                    "pad_b": np.ascontiguousarray(pb_[b]),
                    "wq": _wtap_lhsT(Wtap, qbr, g),
                    "wk": _wtap_lhsT(Wtap, kbr, g),
                    "wv": _wtap_lhsT(Wtap, vbr, g),
                    "bias": np.ascontiguousarray(bias),
                })

    global _last_in_maps
    _last_in_maps = in_maps
    res = run_bass_kernel_spmd(nc, in_maps, list(range(N_CORES)))

    o = np.empty((2, 2, HEADS, T, DH), dtype=np.float32)
    for m in range(2):
        for b in range(2):
            for g in range(2):
                core = m * 4 + b * 2 + g
                blk = res.results[core]["out"].reshape(4, DH, T)
                o[m, b, 4 * g:4 * g + 4] = blk.transpose(0, 2, 1)
    o1 = o[0].reshape(B, T, HEADS * DH)
    o2 = o[1].reshape(B, T, HEADS * DH)
    return o1, o2


# revision 46
# speedup vs baseline: 1.3363x; 1.0303x over previous
"""Trainium2 Bass kernel for nn_CViTFlow (cross-attention ViT flow block).

Math (per the module):
  two token streams x1,x2 [B,T,256] viewed as [B,256,48,48] images.
  6 branches (q1,k1,v1,q2,k2,v2): depthwise3x3 -> BN(eval) -> 1x1 conv -> Linear.
  o1 = softmax(-(q1 k2^T / 16)) v2 + q1 ;  o2 = softmax(-(q2 k1^T / 16)) v1 + q2
  both reshaped [B,H,T,DH] -> [B,T,256] with a plain (head-major) reshape.

Kernel strategy:
  * Host folds BN + 1x1conv + Linear into one 256x256 matrix W and bias c per
    branch, then folds the depthwise 3x3 into 9 "tap" matrices
    Wtap[o,c] = W[o,c]*dw[c,di,dj], so a whole branch becomes 9 shifted
    matmuls accumulated in PSUM (all compute lands on the TensorEngine).
  * Host pre-transposes/pads images to channel-major [256, 50, 50] (zero pad)
    so tap shifts are plain strided access patterns.
  * 8 cores = (map m) x (batch b) x (head-quad g): each core computes one
    output map's 4 heads for one batch element. No collectives.
  * On device: branch matmuls produce qT/kT/vT [128=4*32, T]; v is
    PE-transposed to [t,d] tiles augmented with a ones column (so the AV
    matmul also produces the softmax denominator); scores are computed
    t-chunk-wise with 4 row-tiled (K=32) matmuls sharing the PE array;
    exp on ScalarE; AV accumulates over t in PSUM; finalize divides by the
    denominator (reciprocal + PE broadcast) and adds the q residual.
"""

import numpy as np

B = 2
T = 2304
DIM = 256
HEADS = 8
DH = 32
HW = 48
EPS = 1e-5
P = 128
N_CORES = 8

# t-tiles for the branch phase: row-aligned in the 48x48 image (10/8 rows)
T_TILES = [(0, 480, 0, 10), (480, 480, 10, 10), (960, 480, 20, 10),
           (1440, 480, 30, 10), (1920, 384, 40, 8)]
# l-tiles for the attention phase (256 wide: scores PSUM [128,1024] = 2 banks,
# double-buffered, leaving 4 banks for the col-packed AV accumulators)
NL = 256
N_LT = T // NL  # 9
N_TCH = T // P  # 18 t-chunks of 128 for scores/AV

_PROGRAM = None  # cached Bass program
_last_in_maps = None  # stashed per-core input maps (for external profiling runs)


def _build_program(debug=False):
    """Build the SPMD Bass/Tile program (identical for all 8 cores)."""
    from contextlib import ExitStack

    import concourse.bacc as bacc
    import concourse.mybir as mybir
    import concourse.tile as tile
    from concourse.masks import make_identity
    from concourse.tile_rust import add_dep_helper

    f32 = mybir.dt.float32
    bf16 = mybir.dt.bfloat16
    AF = mybir.ActivationFunctionType
    OP = mybir.AluOpType

    # Bacc (not raw Bass): its compile() runs move_matmul_waits_to_ldweights +
    # generate_event_semaphores, without which walrus rejects multi-wait matmuls
    nc = bacc.Bacc(None, target_bir_lowering=False, debug=False)

    # DRAM I/O (per core).  pad_a feeds the q branch, pad_b feeds k and v.
    # Matmul operands are bf16 (fp32 matmuls cost 2 PE passes via LOW_HIGH
    # mode); accumulation stays fp32 in PSUM.
    pad_a = nc.declare_dram_parameter("pad_a", [2, P, 2500], bf16, isOutput=False)
    pad_b = nc.declare_dram_parameter("pad_b", [2, P, 2500], bf16, isOutput=False)
    wq = nc.declare_dram_parameter("wq", [2, P, 9 * P], bf16, isOutput=False)
    wk = nc.declare_dram_parameter("wk", [2, P, 9 * P], bf16, isOutput=False)
    wv = nc.declare_dram_parameter("wv", [2, P, 9 * P], bf16, isOutput=False)
    bias_d = nc.declare_dram_parameter("bias", [3, P, 1], f32, isOutput=False)
    out_d = nc.declare_dram_parameter("out", [P, T], f32, isOutput=True)

    with tile.TileContext(nc) as tc, ExitStack() as ctx:
        const = ctx.enter_context(tc.tile_pool(name="const", bufs=1))
        sb = ctx.enter_context(tc.tile_pool(name="sb", bufs=1))
        fin = ctx.enter_context(tc.tile_pool(name="fin", bufs=2))

        identity = const.tile([P, P], bf16)
        make_identity(nc, identity)
        ones32 = const.tile([1, 32], mybir.dt.float32r)
        # memset can't target f32r; 1.0 has identical f32/f32r bits
        nc.vector.memset(ones32[:].bitcast(f32), 1.0)

        # ---- input DMAs (k/v weights + image B first: they gate phase A) ----
        wk_sb = sb.tile([P, 2 * 9 * P], bf16, tag="wk")
        pb_sb = sb.tile([P, 2 * 2500], bf16, tag="pb")
        wv_sb = sb.tile([P, 2 * 9 * P], bf16, tag="wv")
        wq_sb = sb.tile([P, 2 * 9 * P], bf16, tag="wq")
        pa_sb = sb.tile([P, 2 * 2500], bf16, tag="pa")
        bias_sb = sb.tile([P, 3], f32, tag="bias")
        for kc in range(2):
            nc.sync.dma_start(wk_sb[:, kc * 1152:(kc + 1) * 1152], wk[kc])
            nc.sync.dma_start(pb_sb[:, kc * 2500:(kc + 1) * 2500], pad_b[kc])
        for kc in range(2):
            nc.sync.dma_start(wv_sb[:, kc * 1152:(kc + 1) * 1152], wv[kc])
        for kc in range(2):
            nc.sync.dma_start(wq_sb[:, kc * 1152:(kc + 1) * 1152], wq[kc])
            nc.sync.dma_start(pa_sb[:, kc * 2500:(kc + 1) * 2500], pad_a[kc])
        for r in range(3):
            nc.sync.dma_start(bias_sb[:, r:r + 1], bias_d[r])

        qT = sb.tile([P, T], bf16, tag="qT")
        qTf = sb.tile([P, T], f32, tag="qTf")   # fp32 copy for the residual
        kT = sb.tile([P, T], bf16, tag="kT")
        vT = sb.tile([P, T], bf16, tag="vT")
        # base-partition-0 copies (head h at cols h*T): score matmuls then all
        # use row strip 0 -> strict-FIFO on PE, so no two concurrent matmul
        # drains ever target the same PSUM bank (same-partition same-bank
        # concurrent drains crash the device)
        qT2 = sb.tile([32, 4 * T], bf16, tag="qT2")
        kT2 = sb.tile([32, 4 * T], bf16, tag="kT2")
        # per t-chunk, per head: 64 cols = [v(32) | ones(1) | zeros(31)] so AV
        # pairs col-tile at (0,0)/(0,64) and the ones column carries the
        # softmax denominator through the same matmul
        vaug = sb.tile([P, N_TCH * 256], bf16, tag="vaug")
        outbuf = sb.tile([P, T], f32, tag="outbuf")

        # ================= Phase A: branch matmuls =================
        def branch(psumA, w_sb, img_sb, dest, role, dest2=None, rebase=None):
            for (t0, nt, r0, nr) in T_TILES:
                ps = psumA.tile([P, nt], f32, tag="sc", bufs=2,
                                name=f"br_{role}_{t0}")
                mm = 0
                for kc in range(2):
                    pv = img_sb[:, kc * 2500:(kc + 1) * 2500].rearrange(
                        "p (r c) -> p r c", c=50)
                    wv_ = w_sb[:, kc * 1152:(kc + 1) * 1152]
                    for di in range(3):
                        for dj in range(3):
                            tap = di * 3 + dj
                            rhs = pv[:, r0 + di:r0 + di + nr, dj:dj + 48]
                            nc.tensor.matmul(
                                ps[:], wv_[:, tap * P:(tap + 1) * P], rhs,
                                start=(mm == 0), stop=(mm == 17))
                            mm += 1
                # bias add, PSUM -> SBUF (bf16 for matmul operands)
                nc.vector.tensor_scalar_add(dest[:, t0:t0 + nt], ps[:],
                                            bias_sb[:, role:role + 1])
                if dest2 is not None:
                    nc.vector.tensor_scalar_add(dest2[:, t0:t0 + nt], ps[:],
                                                bias_sb[:, role:role + 1])
                if rebase is not None:
                    # rebase this slice to partition 0 (head h at cols h*T)
                    # via SBUF->SBUF DMA, spread through the phase so the PE
                    # never has a long DMA-wait gap at the phase transition
                    for h in range(4):
                        nc.sync.dma_start(
                            rebase[0:32, h * T + t0:h * T + t0 + nt],
                            dest[h * 32:(h + 1) * 32, t0:t0 + nt])

        # One PSUM pool for the whole kernel (a pool transition emits barrier
        # boundaries -> a PE idle gap that drops the HAM clock to 1.2 GHz for
        # the rest of the kernel). Branch tiles borrow the "sc" slots, the
        # v-transposes borrow the "avout" slots.
        psumB = ctx.enter_context(tc.tile_pool(name="psum", bufs=2, space="PSUM"))
        ep = ctx.enter_context(tc.tile_pool(name="ep", bufs=2))

        branch(psumB, wk_sb, pb_sb, kT, 1, rebase=kT2)
        branch(psumB, wv_sb, pb_sb, vT, 2)
        branch(psumB, wq_sb, pa_sb, qT, 0, dest2=qTf, rebase=qT2)

        # v: transpose to [t, d] tiles, 64 cols per head
        nc.vector.memset(vaug[:], 0.0)
        for j in range(N_TCH):
            tp = psumB.tile([P, 512], bf16, tag="avout", bufs=2, name=f"tp_{j}")
            nc.tensor.transpose(tp[:, 0:P], vT[:, j * P:(j + 1) * P], identity[:])
            dst = vaug[:, j * 256:(j + 1) * 256].rearrange(
                "p (h c) -> p h c", c=64)[:, :, 0:32]
            src = tp[:, 0:P].rearrange("p (h c) -> p h c", c=32)
            nc.vector.tensor_copy(dst, src)
        ones_cols = vaug.rearrange("p (j h c) -> p j h c", h=4, c=64)[:, :, :, 32:33]
        nc.vector.memset(ones_cols, 1.0)

        # ================= Phase B: attention =================
        # Software-pipelined so the in-order PE stream never sits behind an
        # exp wait: scores(t+1) issue before AV(t); finalize(l-1) is injected
        # mid-way through l's t-loop (its bc matmuls wait on DVE reciprocals).

        sc_tiles = {}

        def scores(li, j):
            l0 = li * NL
            sc = psumB.tile([P, 4 * NL], f32, tag="sc", bufs=2,
                            name=f"sc_{li}_{j}")
            last = None
            for h in range(4):
                last = nc.tensor.matmul(
                    sc[:, h * NL:(h + 1) * NL],
                    kT2[0:32, h * T + j * P:h * T + (j + 1) * P],
                    qT2[0:32, h * T + l0:h * T + l0 + NL],
                    start=True, stop=True)
            sc_tiles[(li, j)] = (sc, last)

        def finalize_recips(li, outp):
            # issue right after the l-tile's AV accumulation completes: the
            # reciprocals (8 cyc/elem on DVE) then overlap the next l-tile's
            # attention instead of head-of-line-blocking the PE at the bc MMs
            recips = []
            for h in range(4):
                cp, pb_ = 256 * (h // 2), 64 * (h % 2)
                recip = fin.tile([1, NL], mybir.dt.float32r, tag="recip",
                                 bufs=8, name=f"recip_{li}_{h}")
                with nc.allow_low_precision(reason="f32r recip: 2^-19 rel "
                                            "is ample for softmax denom"):
                    nc.vector.reciprocal(recip[:],
                                         outp[pb_ + 32:pb_ + 33, cp:cp + NL])
                recips.append(recip)
            return recips

        f32r = mybir.dt.float32r

        def finalize_head(li, outp, recips, h, bc_ps):
            l0 = li * NL
            cp, pb_ = 256 * (h // 2), 64 * (h % 2)
            # float32r: single PE pass (plain fp32 lowers to 2 LOW_HIGH
            # passes); ~2^-19 relative precision is plenty for 1/denom
            nc.tensor.matmul(bc_ps[:, h * NL:(h + 1) * NL],
                             ones32[:], recips[h][:], start=True, stop=True)
            bc_sb = fin.tile([32, NL], f32, tag="bcsb", bufs=4)
            nc.vector.tensor_copy(bc_sb[:], bc_ps[:, h * NL:(h + 1) * NL])
            # av * (1/denom): PSUM+SBUF inputs may differ in base
            # partition (only SB+SB pairs must match), out lands at 32h
            nc.vector.tensor_tensor(outbuf[h * 32:(h + 1) * 32, l0:l0 + NL],
                                    outp[pb_:pb_ + 32, cp:cp + NL],
                                    bc_sb[:], op=OP.mult)
            # in-place residual: both SB inputs at base partition 32h
            nc.vector.tensor_tensor(outbuf[h * 32:(h + 1) * 32, l0:l0 + NL],
                                    outbuf[h * 32:(h + 1) * 32, l0:l0 + NL],
                                    qTf[h * 32:(h + 1) * 32, l0:l0 + NL],
                                    op=OP.add)

        prev = None  # (li, outp) awaiting finalize
        for li in range(N_LT):
            # one bank: pair p at cols 256p, sub s at partitions 64s
            outp = psumB.tile([P, 2 * NL], f32, tag="avout", bufs=2,
                              name=f"avout_{li}")
            if li == 0:
                scores(0, 0)
            for j in range(N_TCH):
                if j + 1 < N_TCH:
                    scores(li, j + 1)
                elif li + 1 < N_LT:
                    scores(li + 1, 0)
                next_last = (sc_tiles[(li, j + 1)][1] if (li, j + 1) in sc_tiles
                             else sc_tiles.get((li + 1, 0), (None, None))[1])
                sc, _ = sc_tiles.pop((li, j))
                et = ep.tile([P, 4 * NL], bf16, tag="e")
                nc.scalar.activation(et[:], sc[:], AF.Exp, scale=-0.0625)
                for h in range(4):
                    cp, sub = 256 * (h // 2), h % 2
                    av = nc.tensor.matmul(
                        outp[64 * sub:64 * sub + 64, cp:cp + NL],
                        vaug[:, j * 256 + 64 * h:j * 256 + 64 * h + 64],
                        et[:, h * NL:(h + 1) * NL],
                        start=(j == 0), stop=(j == N_TCH - 1),
                        tile_position=(0, 64 * sub),
                        skip_group_check=True)
                    if h == 0 and next_last is not None:
                        # keep next scores AHEAD of this exp-gated AV in the
                        # in-order PE stream (scheduling-only ordering edge)
                        add_dep_helper(av.ins, next_last.ins,
                                       reason="scores(t+1) before AV(t)")
                # spread the finalize of l-1 over the middle of this t-loop
                # (one head per chunk, so each PE insertion is small)
                if prev is not None and j in (6, 8, 10, 12):
                    if j == 6:
                        prev_bc = psumB.tile([32, 4 * NL], f32, tag="sc",
                                             bufs=2, name=f"bc_{li}")
                    finalize_head(*prev, (j - 6) // 2, prev_bc)
                    if j == 12:
                        prev = None
            prev = (li, outp, finalize_recips(li, outp))
        last_bc = psumB.tile([32, 4 * NL], f32, tag="sc", bufs=2, name="bc_last")
        for h in range(4):
            finalize_head(*prev, h, last_bc)

        nc.sync.dma_start(out_d[:], outbuf[:])

        if debug:
            for nm, t in [("dbg_qT", qT), ("dbg_kT", kT), ("dbg_vT", vT),
                          ("dbg_vaug", vaug), ("dbg_qTf", qTf)]:
                dd = nc.declare_dram_parameter(nm, list(t.shape), t.dtype,
                                               isOutput=True)
                nc.sync.dma_start(dd[:], t[:])

    nc.compile()
    return nc


def _fold_weights(dw_w, bn_gamma, bn_beta, bn_mean, bn_var, pw_w, pw_b, lin_w):
    """Fold BN + pointwise conv + linear (+ depthwise taps) per branch.

    Returns Wtap [6, 9, 256, 256] (float32) and bias c [6, 256]."""
    dw = dw_w.astype(np.float64)
    g = bn_gamma.astype(np.float64)
    b = bn_beta.astype(np.float64)
    m = bn_mean.astype(np.float64)
    v = bn_var.astype(np.float64)
    pw = pw_w.astype(np.float64)
    pb = pw_b.astype(np.float64)
    lw = lin_w.astype(np.float64)

    scale = g / np.sqrt(v + EPS)                      # [6, 256]
    shift = b - m * scale                             # [6, 256]
    M = np.einsum("noc,ncd->nod", lw, pw)             # lin @ pw  [6, 256, 256]
    W = M * scale[:, None, :]                         # [6, 256(o), 256(c)]
    c = np.einsum("noc,nc->no", M, shift) + np.einsum("noc,nc->no", lw, pb)
    # taps: Wtap[n, di*3+dj, o, c] = W[n, o, c] * dw[n, c, di, dj]
    Wtap = W[:, None, :, :] * dw.transpose(0, 2, 3, 1).reshape(6, 9, 1, 256)
    return Wtap.astype(np.float32), c.astype(np.float32)


def _bf16(a):
    import ml_dtypes
    return a.astype(ml_dtypes.bfloat16)


def _pad_images(x):
    """x [B, T, 256] -> per batch channel-major zero-padded bf16 [2,128,2500]."""
    out = np.zeros((B, 2, P, 50, 50), dtype=np.float32)
    img = np.ascontiguousarray(x.transpose(0, 2, 1)).reshape(B, DIM, HW, HW)
    out[:, :, :, 1:49, 1:49] = img.reshape(B, 2, P, HW, HW)
    return _bf16(out.reshape(B, 2, P, 2500))


def _wtap_lhsT(Wtap, branch, g):
    """Pack lhsT layout [2, 128, 9*128] for a branch restricted to quad g."""
    rows = slice(g * P, (g + 1) * P)
    out = np.empty((2, P, 9 * P), dtype=np.float32)
    for kc in range(2):
        for tap in range(9):
            blk = Wtap[branch, tap][rows, kc * P:(kc + 1) * P]  # [128 o, 128 c]
            out[kc, :, tap * P:(tap + 1) * P] = blk.T
    return _bf16(out)


def kernel(x1, x2, dw_w, bn_gamma, bn_beta, bn_mean, bn_var, pw_w, pw_b, lin_w,
           h1=HW, w1=HW, h2=HW, w2=HW):
    global _PROGRAM
    from concourse.bass_utils import run_bass_kernel_spmd

    x1 = np.asarray(x1, dtype=np.float32)
    x2 = np.asarray(x2, dtype=np.float32)

    Wtap, c = _fold_weights(np.asarray(dw_w), np.asarray(bn_gamma),
                            np.asarray(bn_beta), np.asarray(bn_mean),
                            np.asarray(bn_var), np.asarray(pw_w),
                            np.asarray(pw_b), np.asarray(lin_w))
    pad1 = _pad_images(x1)   # [B, 2, 128, 2500]
    pad2 = _pad_images(x2)

    if _PROGRAM is None:
        _PROGRAM = _build_program()
    nc = _PROGRAM

    # core layout: core = m*4 + b*2 + g
    # map m=0: o1 = att(q=br0(x1), k=br4(x2), v=br5(x2)) + q1
    # map m=1: o2 = att(q=br3(x2), k=br1(x1), v=br2(x1)) + q2
    in_maps = []
    for m in range(2):
        qbr, kbr, vbr = (0, 4, 5) if m == 0 else (3, 1, 2)
        pa, pb_ = (pad1, pad2) if m == 0 else (pad2, pad1)
        for b in range(2):
            for g in range(2):
                bias = np.stack([c[qbr, g * P:(g + 1) * P],
                                 c[kbr, g * P:(g + 1) * P],
                                 c[vbr, g * P:(g + 1) * P]])[:, :, None]
                in_maps.append({
                    "pad_a": np.ascontiguousarray(pa[b]),
                    "pad_b": np.ascontiguousarray(pb_[b]),
                    "wq": _wtap_lhsT(Wtap, qbr, g),
                    "wk": _wtap_lhsT(Wtap, kbr, g),
                    "wv": _wtap_lhsT(Wtap, vbr, g),
                    "bias": np.ascontiguousarray(bias),
                })

    global _last_in_maps
    _last_in_maps = in_maps
    res = run_bass_kernel_spmd(nc, in_maps, list(range(N_CORES)))

    o = np.empty((2, 2, HEADS, T, DH), dtype=np.float32)
    for m in range(2):
        for b in range(2):
            for g in range(2):
                core = m * 4 + b * 2 + g
                blk = res.results[core]["out"].reshape(4, DH, T)
                o[m, b, 4 * g:4 * g + 4] = blk.transpose(0, 2, 1)
    o1 = o[0].reshape(B, T, HEADS * DH)
    o2 = o[1].reshape(B, T, HEADS * DH)
    return o1, o2
